# revision 20
# baseline (speedup 1.0000x reference)
"""CapsuleLayer dynamic-routing kernel for 8 Trainium2 NeuronCores.

Algorithm (validated vs reference in numpy):
  priors P[c,b,n,o] = sum_i x[b,n,i] W[c,n,i,o]; logits are constant along o,
  so routing state is L[c,b,n]. Per routing iteration:
    probs = exp(L)/denom       (softmax over n; no max-subtraction: |L| < ~30)
    s[c,b,o] = sum_n probs*P = (1/denom) sum_{(n,i)} (x*exp(L)) W   <- matmul
    v = squash(s) = s_u * g,  g = nrm/((1+nrm)*sqrt(nrm+eps))/denom
    a[c,b,n] = sum_o P*v = sum_i x * (W^T v)       <- matmul + blockdiag reduce
    L += a
  Sharding: N=1152 split 8 ways (144 route nodes/core); one 174KB AllReduce of
  s-partials + softmax denominators per iteration. Every core computes the
  identical full output; core 0's is returned.
"""

import sys

sys.path.insert(0, "/opt/trn_rl_repo")

import numpy as np
import ml_dtypes

import concourse.bass as bass
import concourse.bacc as bacc
import concourse.mybir as mybir
from concourse import bass_utils
from concourse.tile import TileContext

BF16 = mybir.dt.bfloat16
F32 = mybir.dt.float32
F16 = mybir.dt.float16
AF = mybir.ActivationFunctionType
ALU = mybir.AluOpType

B, N, CI, C, CO = 256, 1152, 8, 10, 16
NCORES = 8
NLOC = N // NCORES          # 144 route nodes per core
K = NLOC * CI               # 1152 local contraction length (n,i)
NCH = K // 128              # 9 partition chunks of (n,i)
NFULL = 128 // CI           # 16 n per chunk
EPS = 1e-8
NITER = 3
CB = C * B                  # 2560


def _build_blockdiag() -> np.ndarray:
    """a-reduce lhsT constants: cols 0..1023 hold 8 [128,128] blocks (chunk j
    maps (n16,i8) row q -> out partition 16j + q//8); cols 1024..1039 hold the
    9th chunk's [128,16] block (out partition q//8)."""
    blk = np.zeros((128, 8 * 128 + 16), np.float32)
    for j in range(8):
        for q in range(128):
            blk[q, 128 * j + 16 * j + q // CI] = 1.0
    for q in range(128):
        blk[q, 1024 + q // CI] = 1.0
    return blk.astype(np.float16)


def _bcast_ap(ap, dim_idx, count):
    """Insert a stride-0 (broadcast) dim into an AP at position dim_idx."""
    dims = [list(d) for d in ap.ap]
    dims.insert(dim_idx, [0, count])
    return bass.AP(tensor=ap.tensor, offset=ap.offset, ap=dims)


def _reshaped_ap(ap, dims):
    return bass.AP(tensor=ap.tensor, offset=ap.offset, ap=[list(d) for d in dims])


def build_kernel():
    nc = bacc.Bacc("TRN2", target_bir_lowering=False, debug=False,
                   num_devices=NCORES)
    xT_d = nc.dram_tensor("xT", [K, B], BF16, kind="ExternalInput")
    xTf_d = nc.dram_tensor("xTf", [K, B], F32, kind="ExternalInput")
    w1_d = nc.dram_tensor("w1", [C, K, CO], BF16, kind="ExternalInput")
    w1r0_d = nc.dram_tensor("w1r0", [128, NCH * 160], BF16,
                            kind="ExternalInput")
    w2_d = nc.dram_tensor("w2", [C, CO, K], F32, kind="ExternalInput")
    blk_d = nc.dram_tensor("blk", [128, 1040], F16, kind="ExternalInput")
    cst_d = nc.dram_tensor("cst", [128, 96], F32, kind="ExternalInput")
    vout_d = nc.dram_tensor("vout", [2, 80, B], F32, kind="ExternalOutput")

    with TileContext(nc) as tc:
        _emit(tc, xT_d.ap(), xTf_d.ap(), w1_d.ap(), w1r0_d.ap(), w2_d.ap(),
              blk_d.ap(), cst_d.ap(), vout_d.ap())
    nc.compile()
    return nc


def _emit(tc, xT_d, xTf_d, w1_d, w1r0_d, w2_d, blk_d, cst_d, vout_d):
    from contextlib import ExitStack
    with ExitStack() as ctx:
        _emit_body(ctx, tc, xT_d, xTf_d, w1_d, w1r0_d, w2_d, blk_d, cst_d,
                   vout_d)


def _emit_body(ctx, tc, xT_d, xTf_d, w1_d, w1r0_d, w2_d, blk_d, cst_d,
               vout_d):
    nc = tc.nc
    state = ctx.enter_context(tc.tile_pool(name="state", bufs=1))
    erep_p = ctx.enter_context(tc.tile_pool(name="erep", bufs=2))
    z_p = ctx.enter_context(tc.tile_pool(name="zp", bufs=2))
    sqp = ctx.enter_context(tc.tile_pool(name="sqp", bufs=2))
    dram = ctx.enter_context(tc.tile_pool(name="dram", bufs=2, space="DRAM"))
    ups_p = ctx.enter_context(tc.tile_pool(name="ups", bufs=1, space="PSUM"))
    acc_p = ctx.enter_context(tc.tile_pool(name="acc", bufs=3, space="PSUM"))
    tiny_p = ctx.enter_context(tc.tile_pool(name="tinyps", bufs=1, space="PSUM"))
    sq_ps = ctx.enter_context(tc.tile_pool(name="sqps", bufs=2, space="PSUM"))

    # ---- persistent SBUF state ----
    xT = state.tile([128, NCH * B], BF16)        # [(n,i) chunk-part, (j, b)]
    xTf = state.tile([128, NCH * B], F32)        # fp32 copy for agreement
    w1 = state.tile([128, C * NCH * CO], BF16)   # s-matmul lhsT blocks
    w2 = state.tile([16, C * K], F32)            # U-matmul lhsT blocks
    blk = state.tile([128, 1040], F16)           # a-reduce lhsT blocks
    ones128 = state.tile([128, 1], BF16)
    ones16f = state.tile([16, 1], F32)
    L = state.tile([128, CB], F32)               # logits, partition = local n
    L9 = state.tile([16, CB], F32)               # local n in [128,144)
    expL = state.tile([128, CB], BF16)
    expL9 = state.tile([16, CB], BF16)
    y_all = state.tile([128, C * NCH * B], BF16)  # y = x*expL per c
    s_part = state.tile([16, CB], F32)
    den_sb = state.tile([1, CB], F32)
    vb = state.tile([16, CB], F32)
    cst = state.tile([128, 96], F32)             # selO5 [80,0:5], selB5 [5,5:85]

    # ---- load inputs / init state ----
    for j in range(NCH):
        nc.sync.dma_start(out=xT[:, j * B:(j + 1) * B],
                          in_=xT_d[j * 128:(j + 1) * 128, :])
        nc.sync.dma_start(out=xTf[:, j * B:(j + 1) * B],
                          in_=xTf_d[j * 128:(j + 1) * 128, :])
    for c in range(C):
        # w1[c] chunk j of 128 (n,i)-rows -> w1 cols (c*NCH+j)*CO .. +CO
        src = w1_d[c].rearrange("(j p) o -> p j o", j=NCH)
        dst = w1[:, c * NCH * CO:(c + 1) * NCH * CO].rearrange(
            "p (j o) -> p j o", j=NCH)
        nc.sync.dma_start(out=dst, in_=src)
    nc.sync.dma_start(out=w2[:].rearrange("p (c k) -> p c k", c=C),
                      in_=w2_d.rearrange("c o k -> o c k"))
    nc.sync.dma_start(out=blk[:], in_=blk_d[:, :])
    nc.sync.dma_start(out=cst[:], in_=cst_d[:, :])
    nc.vector.memset(ones128[:], 1.0)
    nc.vector.memset(ones16f[:], 1.0)
    nc.vector.memset(L[:], 0.0)
    nc.vector.memset(L9[:], 0.0)
    nc.vector.memset(expL[:], 1.0)   # exp(0)
    nc.vector.memset(expL9[:], 1.0)

    HC = C // 2          # capsules per half-collective
    HB = HC * B          # 1280
    # blob rows: [0..16*HC) = s partials [(c,o), b]; [16*HC..16*HC+HC) = denom
    RB = 16 * HC + HC    # 85

    # r0: all-capsule batched s partials (softmax(0) is uniform, so every
    # capsule shares rhs=xT). w1r0 is a host-prepped chunk-major reorder of w1:
    # cols j*160 + (c*16+o), so each chunk j gives a contiguous 128-col
    # (c=0..8) and 32-col (c=8,9) stationary operand.
    sp0 = state.tile([128, B], F32)
    sp1 = state.tile([32, B], F32)
    w1r0 = state.tile([128, NCH * 160], BF16)
    nc.sync.dma_start(out=w1r0[:], in_=w1r0_d[:, :])

    def s_matmuls_r0(blob0, blob1):
        s0a = acc_p.tile([128, B], F32, tag="acc", name="s0a")
        s0b = acc_p.tile([32, B], F32, tag="acc", name="s0b")
        for j in range(NCH):
            rhs = xT[:, j * B:(j + 1) * B]
            nc.tensor.matmul(s0a[:], w1r0[:, j * 160:j * 160 + 128], rhs,
                             start=(j == 0), stop=(j == NCH - 1))
            nc.tensor.matmul(s0b[:], w1r0[:, j * 160 + 128:(j + 1) * 160], rhs,
                             start=(j == 0), stop=(j == NCH - 1))
        nc.scalar.copy(sp0[:], s0a[:])
        nc.scalar.copy(sp1[:], s0b[:])
        nc.sync.dma_start(out=blob0[0:80, :], in_=sp0[0:80, :])
        nc.sync.dma_start(out=blob1[0:48, :], in_=sp0[80:128, :])
        nc.sync.dma_start(out=blob1[48:80, :], in_=sp1[:])

    def s_matmuls(c, it, blob):
        s_ps = acc_p.tile([16, B], F32, tag="acc", name=f"s_ps_{it}_{c}")
        for j in range(NCH):
            rhs = (xT[:, j * B:(j + 1) * B] if it == 0 else
                   y_all[:, (c * NCH + j) * B:(c * NCH + j + 1) * B])
            lo = (c * NCH + j) * CO
            nc.tensor.matmul(s_ps[:], w1[:, lo:lo + CO], rhs,
                             start=(j == 0), stop=(j == NCH - 1))
        nc.scalar.copy(s_part[:, c * B:(c + 1) * B], s_ps[:])
        ch = c % HC
        nc.sync.dma_start(out=blob[16 * ch:16 * ch + 16, :],
                          in_=s_part[:, c * B:(c + 1) * B])

    def den_matmuls(c, it, blob):
        den_ps = tiny_p.tile([1, B], F32, tag="tiny", name=f"den_ps_{it}_{c}")
        nc.tensor.matmul(den_ps[:], ones128[:], expL[:, c * B:(c + 1) * B],
                         start=True, stop=False)
        nc.tensor.matmul(den_ps[:], ones128[0:16, :],
                         expL9[:, c * B:(c + 1) * B],
                         start=False, stop=True)
        nc.scalar.copy(den_sb[0:1, c * B:(c + 1) * B], den_ps[:])
        ch = c % HC
        nc.sync.dma_start(out=blob[16 * HC + ch:16 * HC + ch + 1, :],
                          in_=den_sb[0:1, c * B:(c + 1) * B])

    def collective(blob_in, blob_out):
        nc.gpsimd.collective_compute(
            "AllReduce", ALU.add,
            replica_groups=[list(range(NCORES))],
            ins=[blob_in.opt()], outs=[blob_out.opt()],
        )

    def squash_half(it, h, blob_out, row0=0, const_den=False):
        """v[:, half] = s_u * g for capsules [h*HC, (h+1)*HC)."""
        c0 = h * HC
        # s_u arrives naturally as [(c,o), b]; all squash math stays in that
        # layout. q = sum_o s_u^2 via PE partition-reduce; the eps-free
        # identity v = s_u * sqrt(q) / (den^2 + q) replaces the squash chain;
        # g broadcasts back over o via a tiny PE matmul.
        su = sqp.tile([80, B], F32, tag="su", name=f"su_{it}_{h}")
        nc.sync.dma_start(out=su[:], in_=blob_out[row0:row0 + 16 * HC, :])
        s2t = sqp.tile([80, B], F32, tag="s2t", name=f"s2t_{it}_{h}")
        nc.vector.tensor_mul(s2t[:], su[:], su[:])
        q5 = sq_ps.tile([5, B], F32, tag="sq", name=f"q5_{it}_{h}")
        nc.tensor.matmul(q5[:], cst[0:80, 0:5], s2t[:], start=True, stop=True)
        den5 = sqp.tile([5, B], F32, tag="den5", name=f"den5_{it}_{h}")
        if const_den:
            nc.vector.memset(den5[:], float(N))
        else:
            nc.sync.dma_start(out=den5[:],
                              in_=blob_out[row0 + 16 * HC:row0 + RB, :])
        d2q = sqp.tile([5, B], F32, tag="d2q", name=f"d2q_{it}_{h}")
        nc.vector.tensor_mul(d2q[:], den5[:], den5[:])
        nc.vector.tensor_add(d2q[:], d2q[:], q5[:])
        sqq = sqp.tile([5, B], F32, tag="sqq", name=f"sqq_{it}_{h}")
        nc.scalar.activation(sqq[:], q5[:], AF.Sqrt)
        rr = sqp.tile([5, B], F32, tag="rr", name=f"rr_{it}_{h}")
        nc.vector.reciprocal(rr[:], d2q[:])
        g5 = sqp.tile([5, B], F32, tag="g5", name=f"g5_{it}_{h}")
        nc.vector.tensor_mul(g5[:], sqq[:], rr[:])
        g80 = sq_ps.tile([80, B], F32, tag="sq", name=f"g80_{it}_{h}")
        nc.tensor.matmul(g80[:], cst[0:5, 5:85], g5[:], start=True, stop=True)
        v80 = sqp.tile([80, B], F32, tag="v80", name=f"v80_{it}_{h}")
        nc.vector.tensor_mul(v80[:], su[:], g80[:])
        if it == NITER - 1:
            nc.sync.dma_start(out=vout_d[h], in_=v80[:])
        else:
            for cc in range(HC):
                nc.sync.dma_start(
                    out=vb[:, (c0 + cc) * B:(c0 + cc + 1) * B],
                    in_=v80[16 * cc:16 * cc + 16, :])

    def agreement_update(c):
        z = z_p.tile([128, NCH * B], F16, tag="z", name=f"z_{c}")
        a_ps = acc_p.tile([128, B], F32, tag="acc", name=f"a_ps_{c}")
        a9_ps = acc_p.tile([16, B], F32, tag="acc", name=f"a9_ps_{c}")
        for grp in range(3):
            j0 = 3 * grp
            u_ps = ups_p.tile([128, 3 * B], F32, tag="ups",
                              name=f"u_ps_{c}_{grp}")
            for j in range(j0, j0 + 3):
                lo = c * K + 128 * j
                nc.tensor.matmul(u_ps[:, (j - j0) * B:(j - j0 + 1) * B],
                                 w2[:, lo:lo + 128],
                                 vb[:, c * B:(c + 1) * B],
                                 start=True, stop=True)
            nc.vector.tensor_mul(z[:, j0 * B:(j0 + 3) * B],
                                 xTf[:, j0 * B:(j0 + 3) * B], u_ps[:])
            for j in range(j0, j0 + 3):
                if j < 8:
                    nc.tensor.matmul(a_ps[:], blk[:, 128 * j:128 * (j + 1)],
                                     z[:, j * B:(j + 1) * B],
                                     start=(j == 0), stop=(j == 7))
                else:
                    nc.tensor.matmul(a9_ps[:], blk[:, 1024:1040],
                                     z[:, 8 * B:9 * B], start=True, stop=True)
        nc.vector.tensor_add(L[:, c * B:(c + 1) * B],
                             L[:, c * B:(c + 1) * B], a_ps[:])
        nc.vector.tensor_add(L9[:, c * B:(c + 1) * B],
                             L9[:, c * B:(c + 1) * B], a9_ps[:])
        nc.scalar.activation(expL[:, c * B:(c + 1) * B],
                             L[:, c * B:(c + 1) * B], AF.Exp)
        nc.scalar.activation(expL9[:, c * B:(c + 1) * B],
                             L9[:, c * B:(c + 1) * B], AF.Exp)
        erep = erep_p.tile([128, NCH * B], BF16, tag="erep", name=f"erep_{c}")
        for j in range(NCH):
            s_ap = (expL[16 * j:16 * (j + 1), c * B:(c + 1) * B] if j < 8 else
                    expL9[:, c * B:(c + 1) * B])
            nc.sync.dma_start(out=erep[:, j * B:(j + 1) * B],
                              in_=_bcast_ap(s_ap, 1, CI))
        nc.vector.tensor_mul(y_all[:, c * NCH * B:(c + 1) * NCH * B],
                             xT[:], erep[:])

    # ---- pipelined schedule: half-collectives overlap the other half ----
    blobs = {}
    for r in range(1, NITER):
        for h in range(2):
            blobs[(r, h, "in")] = dram.tile(
                [RB, B], F32, tag=f"bi{r}{h}", name=f"blob_in_{r}_{h}")
            blobs[(r, h, "out")] = dram.tile(
                [RB, B], F32, tag=f"bo{r}{h}", name=f"blob_out_{r}_{h}")
    for h in range(2):
        blobs[(0, h, "in")] = dram.tile(
            [80, B], F32, tag=f"bi0{h}", name=f"blob_in_0_{h}")
        blobs[(0, h, "out")] = dram.tile(
            [80, B], F32, tag=f"bo0{h}", name=f"blob_out_0_{h}")
    warm_in = dram.tile([8, 8], F32, tag="wi", name="warm_in")
    warm_out = dram.tile([8, 8], F32, tag="wo", name="warm_out")

    def work_half(r, h):
        for c in range(h * HC, (h + 1) * HC):
            agreement_update(c)
        for c in range(h * HC, (h + 1) * HC):
            s_matmuls(c, r, blobs[(r, h, "in")])
            den_matmuls(c, r, blobs[(r, h, "in")])
        collective(blobs[(r, h, "in")], blobs[(r, h, "out")])

    # Dummy tiny collective issued first: absorbs the ~11us first-mesh
    # doorbell latency while the input DMAs stream in.
    warm_sb = state.tile([8, 8], F32)
    nc.vector.memset(warm_sb[:], 0.0)
    nc.sync.dma_start(out=warm_in[:, :], in_=warm_sb[:])
    collective(warm_in, warm_out)

    # r=0: probs are uniform (softmax of zero logits) -> batched matmuls over
    # all capsules; denominators known to be exactly N=1152.
    s_matmuls_r0(blobs[(0, 0, "in")], blobs[(0, 1, "in")])
    collective(blobs[(0, 0, "in")], blobs[(0, 0, "out")])
    collective(blobs[(0, 1, "in")], blobs[(0, 1, "out")])
    squash_half(0, 0, blobs[(0, 0, "out")], row0=0, const_den=True)
    squash_half(0, 1, blobs[(0, 1, "out")], row0=0, const_den=True)
    for r in range(1, NITER):
        work_half(r, 0)
        squash_half(r, 0, blobs[(r, 0, "out")])   # overlaps work_half(r,1) PE
        work_half(r, 1)
        squash_half(r, 1, blobs[(r, 1, "out")])   # overlaps work_half(r+1,0)


def _build_cst() -> np.ndarray:
    """Squash constants: selO5 (o-sum per capsule) and selB5 (o-broadcast)."""
    cst = np.zeros((128, 96), np.float32)
    for cc in range(5):
        for o in range(CO):
            cst[16 * cc + o, cc] = 1.0          # selO5 [80, 0:5]
            cst[cc, 5 + 16 * cc + o] = 1.0      # selB5 [5, 5:85]
    return cst


def _prep_inputs(x: np.ndarray, route_weights: np.ndarray):
    """Host-side sharding + layout prep. Returns per-core input maps."""
    bf = ml_dtypes.bfloat16
    blk = _build_blockdiag()
    cst = _build_cst()
    in_maps = []
    for k in range(NCORES):
        sl = slice(k * NLOC, (k + 1) * NLOC)
        xT = np.ascontiguousarray(
            x[:, sl, :].transpose(1, 2, 0).reshape(K, B)).astype(bf)
        w1 = np.ascontiguousarray(
            route_weights[:, sl].reshape(C, K, CO)).astype(bf)
        w1f = np.ascontiguousarray(
            route_weights[:, sl].reshape(C, K, CO)).astype(np.float32)
        w2 = np.ascontiguousarray(w1f.transpose(0, 2, 1)).astype(np.float32)
        # chunk-major batched-lhsT layout: [p, j*160 + c*16 + o]
        w1r0 = np.ascontiguousarray(
            w1.reshape(C, NCH, 128, CO).transpose(2, 1, 0, 3).reshape(
                128, NCH * 160))
        xTf = np.ascontiguousarray(
            x[:, sl, :].transpose(1, 2, 0).reshape(K, B)).astype(np.float32)
        in_maps.append({"xT": xT, "xTf": xTf, "w1": w1, "w1r0": w1r0,
                       "w2": w2, "blk": blk, "cst": cst})
    return in_maps


_NC_CACHE = {}


def _get_nc():
    if "nc" not in _NC_CACHE:
        _NC_CACHE["nc"] = build_kernel()
    return _NC_CACHE["nc"]


def _postprocess(v: np.ndarray) -> np.ndarray:
    # v: [2, 80, B] with rows (c', o) per half -> [C, B, 1, 1, O]
    out = v.reshape(2, 5, CO, B).transpose(0, 1, 3, 2).reshape(C, B, 1, 1, CO)
    return np.ascontiguousarray(out.astype(np.float32))


def kernel(x: np.ndarray, route_weights: np.ndarray) -> np.ndarray:
    nc = _get_nc()
    in_maps = _prep_inputs(np.asarray(x, np.float32),
                           np.asarray(route_weights, np.float32))
    res = bass_utils.run_bass_kernel_spmd(nc, in_maps,
                                          core_ids=list(range(NCORES)))
    return _postprocess(np.asarray(res.results[0]["vout"], np.float32))


def kernel_sim(x: np.ndarray, route_weights: np.ndarray) -> np.ndarray:
    """CoreSim (multi-core simulator) path for correctness debugging."""
    from concourse.bass_interp import MultiCoreSim
    nc = _get_nc()
    in_maps = _prep_inputs(np.asarray(x, np.float32),
                           np.asarray(route_weights, np.float32))
    sim = MultiCoreSim(nc, num_cores=NCORES)
    for i, core in sim.cores.items():
        for name, arr in in_maps[i].items():
            core.tensor(name)[:] = arr
    sim.simulate(check_with_hw=False)
    return _postprocess(np.asarray(sim.cores[0].tensor("vout"), np.float32))



# revision 21
# speedup vs baseline: 1.3718x; 1.3718x over previous
"""CapsuleLayer dynamic-routing kernel for 8 Trainium2 NeuronCores.

Algorithm (validated vs reference in numpy):
  priors P[c,b,n,o] = sum_i x[b,n,i] W[c,n,i,o]; logits are constant along o,
  so routing state is L[c,b,n]. Per routing iteration:
    probs = exp(L)/denom       (softmax over n; no max-subtraction: |L| < ~30)
    s[c,b,o] = sum_n probs*P = (1/denom) sum_{(n,i)} (x*exp(L)) W   <- matmul
    v = squash(s) = s_u * g,  g = nrm/((1+nrm)*sqrt(nrm+eps))/denom
    a[c,b,n] = sum_o P*v = sum_i x * (W^T v)       <- matmul + blockdiag reduce
    L += a
  Sharding: N=1152 split 8 ways (144 route nodes/core); one 174KB AllReduce of
  s-partials + softmax denominators per iteration. Every core computes the
  identical full output; core 0's is returned.
"""

import sys

sys.path.insert(0, "/opt/trn_rl_repo")

import numpy as np
import ml_dtypes

import concourse.bass as bass
import concourse.bacc as bacc
import concourse.mybir as mybir
from concourse import bass_utils
from concourse.tile import TileContext

BF16 = mybir.dt.bfloat16
F32 = mybir.dt.float32
F16 = mybir.dt.float16
AF = mybir.ActivationFunctionType
ALU = mybir.AluOpType

B, N, CI, C, CO = 256, 1152, 8, 10, 16
NCORES = 8
NLOC = N // NCORES          # 144 route nodes per core
K = NLOC * CI               # 1152 local contraction length (n,i)
NCH = K // 128              # 9 partition chunks of (n,i)
NFULL = 128 // CI           # 16 n per chunk
EPS = 1e-8
NITER = 3
CB = C * B                  # 2560


def _build_blockdiag() -> np.ndarray:
    """a-reduce lhsT constants: cols 0..1023 hold 8 [128,128] blocks (chunk j
    maps (n16,i8) row q -> out partition 16j + q//8); cols 1024..1039 hold the
    9th chunk's [128,16] block (out partition q//8)."""
    blk = np.zeros((128, 8 * 128 + 16), np.float32)
    for j in range(8):
        for q in range(128):
            blk[q, 128 * j + 16 * j + q // CI] = 1.0
    for q in range(128):
        blk[q, 1024 + q // CI] = 1.0
    return blk.astype(np.float16)


def _bcast_ap(ap, dim_idx, count):
    """Insert a stride-0 (broadcast) dim into an AP at position dim_idx."""
    dims = [list(d) for d in ap.ap]
    dims.insert(dim_idx, [0, count])
    return bass.AP(tensor=ap.tensor, offset=ap.offset, ap=dims)


def _reshaped_ap(ap, dims):
    return bass.AP(tensor=ap.tensor, offset=ap.offset, ap=[list(d) for d in dims])


def build_kernel():
    nc = bacc.Bacc("TRN2", target_bir_lowering=False, debug=False,
                   num_devices=NCORES)
    xT_d = nc.dram_tensor("xT", [K, B], BF16, kind="ExternalInput")
    xTf_d = nc.dram_tensor("xTf", [K, B], F32, kind="ExternalInput")
    w1_d = nc.dram_tensor("w1", [128, C * NCH * CO], BF16,
                           kind="ExternalInput")
    w1r0_d = nc.dram_tensor("w1r0", [128, NCH * 160], BF16,
                            kind="ExternalInput")
    w2_d = nc.dram_tensor("w2", [16, C * K], F16, kind="ExternalInput")
    blk_d = nc.dram_tensor("blk", [128, 1040], F16, kind="ExternalInput")
    cst_d = nc.dram_tensor("cst", [128, 96], F32, kind="ExternalInput")
    vout_d = nc.dram_tensor("vout", [2, 80, B], F32, kind="ExternalOutput")

    with TileContext(nc) as tc:
        _emit(tc, xT_d.ap(), xTf_d.ap(), w1_d.ap(), w1r0_d.ap(), w2_d.ap(),
              blk_d.ap(), cst_d.ap(), vout_d.ap())
    nc.compile()
    return nc


def _emit(tc, xT_d, xTf_d, w1_d, w1r0_d, w2_d, blk_d, cst_d, vout_d):
    from contextlib import ExitStack
    with ExitStack() as ctx:
        _emit_body(ctx, tc, xT_d, xTf_d, w1_d, w1r0_d, w2_d, blk_d, cst_d,
                   vout_d)


def _emit_body(ctx, tc, xT_d, xTf_d, w1_d, w1r0_d, w2_d, blk_d, cst_d,
               vout_d):
    nc = tc.nc
    state = ctx.enter_context(tc.tile_pool(name="state", bufs=1))
    erep_p = ctx.enter_context(tc.tile_pool(name="erep", bufs=2))
    z_p = ctx.enter_context(tc.tile_pool(name="zp", bufs=2))
    sqp = ctx.enter_context(tc.tile_pool(name="sqp", bufs=2))
    dram = ctx.enter_context(tc.tile_pool(name="dram", bufs=2, space="DRAM"))
    ups_p = ctx.enter_context(tc.tile_pool(name="ups", bufs=2, space="PSUM"))
    acc_p = ctx.enter_context(tc.tile_pool(name="acc", bufs=3, space="PSUM"))
    tiny_p = ctx.enter_context(tc.tile_pool(name="tinyps", bufs=1, space="PSUM"))
    sq_ps = ctx.enter_context(tc.tile_pool(name="sqps", bufs=2, space="PSUM"))

    # ---- persistent SBUF state ----
    xT = state.tile([128, NCH * B], BF16)        # [(n,i) chunk-part, (j, b)]
    xTf = state.tile([128, NCH * B], F32)        # fp32 copy for agreement
    w1 = state.tile([128, C * NCH * CO], BF16)   # s-matmul lhsT blocks
    w2 = state.tile([16, C * K], F16)            # U-matmul lhsT blocks
    blk = state.tile([128, 1040], F16)           # a-reduce lhsT blocks
    ones128 = state.tile([128, 1], BF16)
    ones16f = state.tile([16, 1], F32)
    L = state.tile([128, CB], F32)               # logits, partition = local n
    L9 = state.tile([16, CB], F32)               # local n in [128,144)
    expL = state.tile([128, CB], BF16)
    expL9 = state.tile([16, CB], BF16)
    y_all = state.tile([128, C * NCH * B], BF16)  # y = x*expL per c
    s_part = state.tile([16, CB], F32)
    den_sb = state.tile([1, CB], F32)
    vb = state.tile([16, CB], F16)
    cst = state.tile([128, 96], F32)             # selO5 [80,0:5], selB5 [5,5:85]

    # ---- load inputs / init state (xT first: r0 matmuls need only these) ----
    for j in range(NCH):
        nc.sync.dma_start(out=xT[:, j * B:(j + 1) * B],
                          in_=xT_d[j * 128:(j + 1) * 128, :])
    nc.sync.dma_start(out=cst[:], in_=cst_d[:, :])
    nc.sync.dma_start(out=w1[:], in_=w1_d[:, :])
    nc.sync.dma_start(out=w2[:], in_=w2_d[:, :])
    nc.sync.dma_start(out=blk[:], in_=blk_d[:, :])
    for j in range(NCH):
        nc.sync.dma_start(out=xTf[:, j * B:(j + 1) * B],
                          in_=xTf_d[j * 128:(j + 1) * 128, :])
    nc.vector.memset(ones128[:], 1.0)
    nc.vector.memset(ones16f[:], 1.0)
    nc.vector.memset(L[:], 0.0)
    nc.vector.memset(L9[:], 0.0)
    nc.vector.memset(expL[:], 1.0)   # exp(0)
    nc.vector.memset(expL9[:], 1.0)

    HC = C // 2          # capsules per half-collective
    HB = HC * B          # 1280
    # blob rows: [0..16*HC) = s partials [(c,o), b]; [16*HC..16*HC+HC) = denom
    RB = 16 * HC + HC    # 85

    # r0: all-capsule batched s partials (softmax(0) is uniform, so every
    # capsule shares rhs=xT). w1r0 is a host-prepped chunk-major reorder of w1:
    # cols j*160 + (c*16+o), so each chunk j gives a contiguous 128-col
    # (c=0..8) and 32-col (c=8,9) stationary operand.
    sp0 = state.tile([128, B], F32)
    sp1 = state.tile([32, B], F32)
    w1r0 = state.tile([128, NCH * 160], BF16)
    nc.sync.dma_start(out=w1r0[:], in_=w1r0_d[:, :])

    def s_matmuls_r0(blob0, blob1):
        s0a = acc_p.tile([128, B], F32, tag="acc", name="s0a")
        s0b = acc_p.tile([32, B], F32, tag="acc", name="s0b")
        for j in range(NCH):
            rhs = xT[:, j * B:(j + 1) * B]
            nc.tensor.matmul(s0a[:], w1r0[:, j * 160:j * 160 + 128], rhs,
                             start=(j == 0), stop=(j == NCH - 1))
            nc.tensor.matmul(s0b[:], w1r0[:, j * 160 + 128:(j + 1) * 160], rhs,
                             start=(j == 0), stop=(j == NCH - 1))
        nc.scalar.copy(sp0[:], s0a[:])
        nc.scalar.copy(sp1[:], s0b[:])
        nc.sync.dma_start(out=blob0[0:80, :], in_=sp0[0:80, :])
        nc.sync.dma_start(out=blob1[0:48, :], in_=sp0[80:128, :])
        nc.sync.dma_start(out=blob1[48:80, :], in_=sp1[:])

    def s_matmuls(c, it, blob):
        s_ps = acc_p.tile([16, B], F32, tag="acc", name=f"s_ps_{it}_{c}")
        for j in range(NCH):
            rhs = (xT[:, j * B:(j + 1) * B] if it == 0 else
                   y_all[:, (c * NCH + j) * B:(c * NCH + j + 1) * B])
            lo = (c * NCH + j) * CO
            nc.tensor.matmul(s_ps[:], w1[:, lo:lo + CO], rhs,
                             start=(j == 0), stop=(j == NCH - 1))
        nc.scalar.copy(s_part[:, c * B:(c + 1) * B], s_ps[:])
        ch = c % HC
        nc.sync.dma_start(out=blob[16 * ch:16 * ch + 16, :],
                          in_=s_part[:, c * B:(c + 1) * B])

    def den_matmuls(c, it, blob):
        den_ps = tiny_p.tile([1, B], F32, tag="tiny", name=f"den_ps_{it}_{c}")
        nc.tensor.matmul(den_ps[:], ones128[:], expL[:, c * B:(c + 1) * B],
                         start=True, stop=False)
        nc.tensor.matmul(den_ps[:], ones128[0:16, :],
                         expL9[:, c * B:(c + 1) * B],
                         start=False, stop=True)
        nc.scalar.copy(den_sb[0:1, c * B:(c + 1) * B], den_ps[:])
        ch = c % HC
        nc.sync.dma_start(out=blob[16 * HC + ch:16 * HC + ch + 1, :],
                          in_=den_sb[0:1, c * B:(c + 1) * B])

    def collective(blob_in, blob_out):
        nc.gpsimd.collective_compute(
            "AllReduce", ALU.add,
            replica_groups=[list(range(NCORES))],
            ins=[blob_in.opt()], outs=[blob_out.opt()],
        )

    def squash_half(it, h, blob_out, row0=0, const_den=False):
        """v[:, half] = s_u * g for capsules [h*HC, (h+1)*HC)."""
        c0 = h * HC
        # s_u arrives naturally as [(c,o), b]; all squash math stays in that
        # layout. q = sum_o s_u^2 via PE partition-reduce; the eps-free
        # identity v = s_u * sqrt(q) / (den^2 + q) replaces the squash chain;
        # g broadcasts back over o via a tiny PE matmul.
        su = sqp.tile([80, B], F32, tag="su", name=f"su_{it}_{h}")
        nc.sync.dma_start(out=su[:], in_=blob_out[row0:row0 + 16 * HC, :])
        s2t = sqp.tile([80, B], F32, tag="s2t", name=f"s2t_{it}_{h}")
        nc.vector.tensor_mul(s2t[:], su[:], su[:])
        q5 = sq_ps.tile([5, B], F32, tag="sq", name=f"q5_{it}_{h}")
        nc.tensor.matmul(q5[:], cst[0:80, 0:5], s2t[:], start=True, stop=True)
        den5 = sqp.tile([5, B], F32, tag="den5", name=f"den5_{it}_{h}")
        if const_den:
            nc.vector.memset(den5[:], float(N))
        else:
            nc.sync.dma_start(out=den5[:],
                              in_=blob_out[row0 + 16 * HC:row0 + RB, :])
        d2q = sqp.tile([5, B], F32, tag="d2q", name=f"d2q_{it}_{h}")
        nc.vector.tensor_mul(d2q[:], den5[:], den5[:])
        nc.vector.tensor_add(d2q[:], d2q[:], q5[:])
        sqq = sqp.tile([5, B], F32, tag="sqq", name=f"sqq_{it}_{h}")
        nc.scalar.activation(sqq[:], q5[:], AF.Sqrt)
        rr = sqp.tile([5, B], F32, tag="rr", name=f"rr_{it}_{h}")
        nc.vector.reciprocal(rr[:], d2q[:])
        g5 = sqp.tile([5, B], F32, tag="g5", name=f"g5_{it}_{h}")
        nc.vector.tensor_mul(g5[:], sqq[:], rr[:])
        g80 = sq_ps.tile([80, B], F32, tag="sq", name=f"g80_{it}_{h}")
        nc.tensor.matmul(g80[:], cst[0:5, 5:85], g5[:], start=True, stop=True)
        v80 = sqp.tile([80, B], F32, tag="v80", name=f"v80_{it}_{h}")
        nc.vector.tensor_mul(v80[:], su[:], g80[:])
        if it == NITER - 1:
            nc.sync.dma_start(out=vout_d[h], in_=v80[:])
        else:
            v80h = sqp.tile([80, B], F16, tag="v80h", name=f"v80h_{it}_{h}")
            nc.scalar.copy(v80h[:], v80[:])
            for cc in range(HC):
                nc.sync.dma_start(
                    out=vb[:, (c0 + cc) * B:(c0 + cc + 1) * B],
                    in_=v80h[16 * cc:16 * cc + 16, :])

    def agreement_update(c):
        z = z_p.tile([128, NCH * B], F16, tag="z", name=f"z_{c}")
        a_ps = acc_p.tile([128, B], F32, tag="acc", name=f"a_ps_{c}")
        a9_ps = acc_p.tile([16, B], F32, tag="acc", name=f"a9_ps_{c}")
        for grp in range(5):
            j0 = 2 * grp
            nj = 2 if grp < 4 else 1
            u_ps = ups_p.tile([128, 2 * B], F32, tag="ups",
                              name=f"u_ps_{c}_{grp}")
            for j in range(j0, j0 + nj):
                lo = c * K + 128 * j
                nc.tensor.matmul(u_ps[:, (j - j0) * B:(j - j0 + 1) * B],
                                 w2[:, lo:lo + 128],
                                 vb[:, c * B:(c + 1) * B],
                                 start=True, stop=True)
            nc.vector.tensor_mul(z[:, j0 * B:(j0 + nj) * B],
                                 xTf[:, j0 * B:(j0 + nj) * B],
                                 u_ps[:, 0:nj * B])
            for j in range(j0, j0 + nj):
                if j < 8:
                    nc.tensor.matmul(a_ps[:], blk[:, 128 * j:128 * (j + 1)],
                                     z[:, j * B:(j + 1) * B],
                                     start=(j == 0), stop=(j == 7))
                else:
                    nc.tensor.matmul(a9_ps[:], blk[:, 1024:1040],
                                     z[:, 8 * B:9 * B], start=True, stop=True)
        nc.vector.tensor_add(L[:, c * B:(c + 1) * B],
                             L[:, c * B:(c + 1) * B], a_ps[:])
        nc.vector.tensor_add(L9[:, c * B:(c + 1) * B],
                             L9[:, c * B:(c + 1) * B], a9_ps[:])
        nc.scalar.activation(expL[:, c * B:(c + 1) * B],
                             L[:, c * B:(c + 1) * B], AF.Exp)
        nc.scalar.activation(expL9[:, c * B:(c + 1) * B],
                             L9[:, c * B:(c + 1) * B], AF.Exp)
        erep = erep_p.tile([128, NCH * B], BF16, tag="erep", name=f"erep_{c}")
        for j in range(NCH):
            s_ap = (expL[16 * j:16 * (j + 1), c * B:(c + 1) * B] if j < 8 else
                    expL9[:, c * B:(c + 1) * B])
            nc.sync.dma_start(out=erep[:, j * B:(j + 1) * B],
                              in_=_bcast_ap(s_ap, 1, CI))
        nc.vector.tensor_mul(y_all[:, c * NCH * B:(c + 1) * NCH * B],
                             xT[:], erep[:])

    # ---- pipelined schedule: half-collectives overlap the other half ----
    blobs = {}
    for r in range(1, NITER):
        for h in range(2):
            blobs[(r, h, "in")] = dram.tile(
                [RB, B], F32, tag=f"bi{r}{h}", name=f"blob_in_{r}_{h}")
            blobs[(r, h, "out")] = dram.tile(
                [RB, B], F32, tag=f"bo{r}{h}", name=f"blob_out_{r}_{h}")
    for h in range(2):
        blobs[(0, h, "in")] = dram.tile(
            [80, B], F32, tag=f"bi0{h}", name=f"blob_in_0_{h}")
        blobs[(0, h, "out")] = dram.tile(
            [80, B], F32, tag=f"bo0{h}", name=f"blob_out_0_{h}")
    warm_in = dram.tile([8, 8], F32, tag="wi", name="warm_in")
    warm_out = dram.tile([8, 8], F32, tag="wo", name="warm_out")

    def work_half(r, h):
        for c in range(h * HC, (h + 1) * HC):
            agreement_update(c)
        for c in range(h * HC, (h + 1) * HC):
            s_matmuls(c, r, blobs[(r, h, "in")])
            den_matmuls(c, r, blobs[(r, h, "in")])
        collective(blobs[(r, h, "in")], blobs[(r, h, "out")])

    # Dummy tiny collective issued first: absorbs the ~11us first-mesh
    # doorbell latency while the input DMAs stream in.
    warm_sb = state.tile([8, 8], F32)
    nc.vector.memset(warm_sb[:], 0.0)
    nc.sync.dma_start(out=warm_in[:, :], in_=warm_sb[:])
    collective(warm_in, warm_out)

    # r=0: probs are uniform (softmax of zero logits) -> batched matmuls over
    # all capsules; denominators known to be exactly N=1152.
    s_matmuls_r0(blobs[(0, 0, "in")], blobs[(0, 1, "in")])
    collective(blobs[(0, 0, "in")], blobs[(0, 0, "out")])
    collective(blobs[(0, 1, "in")], blobs[(0, 1, "out")])
    squash_half(0, 0, blobs[(0, 0, "out")], row0=0, const_den=True)
    squash_half(0, 1, blobs[(0, 1, "out")], row0=0, const_den=True)
    for r in range(1, NITER):
        work_half(r, 0)
        squash_half(r, 0, blobs[(r, 0, "out")])   # overlaps work_half(r,1) PE
        work_half(r, 1)
        squash_half(r, 1, blobs[(r, 1, "out")])   # overlaps work_half(r+1,0)


def _build_cst() -> np.ndarray:
    """Squash constants: selO5 (o-sum per capsule) and selB5 (o-broadcast)."""
    cst = np.zeros((128, 96), np.float32)
    for cc in range(5):
        for o in range(CO):
            cst[16 * cc + o, cc] = 1.0          # selO5 [80, 0:5]
            cst[cc, 5 + 16 * cc + o] = 1.0      # selB5 [5, 5:85]
    return cst


def _prep_inputs(x: np.ndarray, route_weights: np.ndarray):
    """Host-side sharding + layout prep. Returns per-core input maps."""
    bf = ml_dtypes.bfloat16
    blk = _build_blockdiag()
    cst = _build_cst()
    in_maps = []
    for k in range(NCORES):
        sl = slice(k * NLOC, (k + 1) * NLOC)
        xT = np.ascontiguousarray(
            x[:, sl, :].transpose(1, 2, 0).reshape(K, B)).astype(bf)
        w1c = np.ascontiguousarray(
            route_weights[:, sl].reshape(C, K, CO)).astype(bf)
        w1f = np.ascontiguousarray(
            route_weights[:, sl].reshape(C, K, CO)).astype(np.float32)
        # w1 in SBUF layout [p, (c, j, o)]
        w1 = np.ascontiguousarray(
            w1c.reshape(C, NCH, 128, CO).transpose(2, 0, 1, 3).reshape(
                128, C * NCH * CO))
        # w2 in SBUF layout [o, (c, k)]
        w2 = np.ascontiguousarray(
            w1f.transpose(2, 0, 1).reshape(CO, C * K)).astype(np.float16)
        # chunk-major batched-lhsT layout: [p, j*160 + c*16 + o]
        w1r0 = np.ascontiguousarray(
            w1c.reshape(C, NCH, 128, CO).transpose(2, 1, 0, 3).reshape(
                128, NCH * 160))
        xTf = np.ascontiguousarray(
            x[:, sl, :].transpose(1, 2, 0).reshape(K, B)).astype(np.float32)
        in_maps.append({"xT": xT, "xTf": xTf, "w1": w1, "w1r0": w1r0,
                       "w2": w2, "blk": blk, "cst": cst})
    return in_maps


_NC_CACHE = {}


def _get_nc():
    if "nc" not in _NC_CACHE:
        _NC_CACHE["nc"] = build_kernel()
    return _NC_CACHE["nc"]


def _postprocess(v: np.ndarray) -> np.ndarray:
    # v: [2, 80, B] with rows (c', o) per half -> [C, B, 1, 1, O]
    out = v.reshape(2, 5, CO, B).transpose(0, 1, 3, 2).reshape(C, B, 1, 1, CO)
    return np.ascontiguousarray(out.astype(np.float32))


def kernel(x: np.ndarray, route_weights: np.ndarray) -> np.ndarray:
    nc = _get_nc()
    in_maps = _prep_inputs(np.asarray(x, np.float32),
                           np.asarray(route_weights, np.float32))
    res = bass_utils.run_bass_kernel_spmd(nc, in_maps,
                                          core_ids=list(range(NCORES)))
    return _postprocess(np.asarray(res.results[0]["vout"], np.float32))


def kernel_sim(x: np.ndarray, route_weights: np.ndarray) -> np.ndarray:
    """CoreSim (multi-core simulator) path for correctness debugging."""
    from concourse.bass_interp import MultiCoreSim
    nc = _get_nc()
    in_maps = _prep_inputs(np.asarray(x, np.float32),
                           np.asarray(route_weights, np.float32))
    sim = MultiCoreSim(nc, num_cores=NCORES)
    for i, core in sim.cores.items():
        for name, arr in in_maps[i].items():
            core.tensor(name)[:] = arr
    sim.simulate(check_with_hw=False)
    return _postprocess(np.asarray(sim.cores[0].tensor("vout"), np.float32))

